# revision 7
# baseline (speedup 1.0000x reference)
"""Trainium2 Bass kernel for the BiPartialTestBlock GNN message-passing layer.

Math (per batch g):
    Lmsg   = MLP_Lm(Ls)                                  [nL, d]
    MTLmsg = Ms^T @ Lmsg                                 [nC, d]
    cu     = Cs + tanh(MLP_Cu([Cs, MTLmsg]))             [nC, d]
    Cmsg   = MLP_Cm(cu)                                  [nC, d]
    MCmsg  = Ms @ Cmsg                                   [nL, d]
    lu     = Ls + tanh(MLP_Lu([Ls, FL, MCmsg]))          [nL, d]
returns (lu, cu).

Sharding (8 cores): core c -> (g = c // 4) batch, (i = c % 4) nL-shard of
4096 rows. Pass 1 contracts over each core's nL shard (partial MTLmsg),
AllReduce'd over the 4-core group in 4 pipelined nC chunks. cu/Cmsg are
recomputed (replicated) per core; pass 2 uses a host-transposed copy of the
core's own Ms block, so both big matmuls stream natural-layout [128, 2048]
f32 tiles at full DMA width. All activations live feature-major ([d, tok]),
which makes every MLP a weights-stationary matmul and the two concats pure
partition-stacking.
"""

import sys

sys.path.insert(0, "/opt/trn_rl_repo")

import numpy as np

import concourse.bass as bass  # noqa: F401  (AP types via bacc)
import concourse.mybir as mybir
import concourse.tile as tile
from concourse import bacc
from concourse.bass_utils import run_bass_kernel_spmd

N_CORES = 8
B, NL, NC, D, H = 2, 16384, 8192, 64, 128
SH = NL // 4  # per-core literal shard

F32 = mybir.dt.float32
F32R = mybir.dt.float32r
AF = mybir.ActivationFunctionType

# "f32r" runs the two adjacency matmuls in reduced-precision fp32 (1 PE
# cycle/row instead of 4); "f32" is full precision.
BIG_DT = "f32r"
TRACE = False
TRACE_DIR = None
LAST_RESULT = None

_CACHE = {}


def build_nc(big_dt=BIG_DT, sh=SH, ncdim=NC, c_chunk=2048, l_chunk=2048):
    """Emit the SPMD Bass module (identical program on all 8 cores)."""
    c_chunk = min(c_chunk, ncdim)
    l_chunk = min(l_chunk, sh)
    LO = sh // 128            # pass-1 l-tiles (contraction)
    CCN = ncdim // c_chunk    # pass-1 / allreduce chunks
    NJ = c_chunk // 512       # psum banks per pass-1 chunk
    CO = ncdim // 128         # pass-2 c-tiles (contraction)
    LCN = sh // l_chunk       # pass-2 chunks
    LJ = l_chunk // 512
    QN = ncdim // 512         # cu/Cmsg sub-chunks
    BQ = c_chunk // 512       # stage-D sub-chunks per allreduce chunk
    SB = sh // 512            # Lmsg sub-chunks

    BD = F32R if big_dt == "f32r" else F32

    def mm(ap):
        return ap

    nc = bacc.Bacc("TRN2", target_bir_lowering=False, debug=False,
                   enable_asserts=False, num_devices=N_CORES)

    def din(name, shape):
        return nc.dram_tensor(name, shape, F32, kind="ExternalInput").ap()

    ms = nc.dram_tensor("ms", [sh, ncdim], BD, kind="ExternalInput").ap()
    mst = nc.dram_tensor("mst", [ncdim, sh], BD, kind="ExternalInput").ap()
    lsfl = din("lsfl", [128, sh])
    cst = din("cst", [64, ncdim])
    lm_w1s = din("lm_w1s", [128, 128])
    lm_b1 = din("lm_b1", [128, 1])
    lm_w2 = din("lm_w2", [128, 64])
    lm_b2r = din("lm_b2r", [128, 64])
    cm_w1s = din("cm_w1s", [128, 128])
    cm_b1 = din("cm_b1", [128, 1])
    cm_w2 = din("cm_w2", [128, 64])
    cm_b2r = din("cm_b2r", [128, 64])
    cu_w1 = din("cu_w1", [128, 128])
    cu_b1 = din("cu_b1", [128, 1])
    cu_w2 = din("cu_w2", [128, 64])
    cu_b2 = din("cu_b2", [64, 1])
    lu_w1a = din("lu_w1a", [128, 128])
    lu_w1b = din("lu_w1b", [128, 128])
    lu_b1 = din("lu_b1", [128, 1])
    lu_w2 = din("lu_w2", [128, 64])
    lu_b2 = din("lu_b2", [64, 1])

    lut_o = nc.dram_tensor("lut", [64, sh], F32, kind="ExternalOutput").ap()
    cut_o = nc.dram_tensor("cut", [64, ncdim], F32, kind="ExternalOutput").ap()

    with tile.TileContext(nc) as tc:
        with tc.tile_pool(name="pers", bufs=1) as pers, \
             tc.tile_pool(name="stream", bufs=5) as stream, \
             tc.tile_pool(name="work", bufs=3) as work, \
             tc.tile_pool(name="accp", bufs=1, space="PSUM") as accp, \
             tc.tile_pool(name="mlpp", bufs=2, space="PSUM") as mlpp, \
             tc.tile_pool(name="dram", bufs=1, space="DRAM") as dram:

            # ---- prologue: small inputs -> SBUF -------------------------
            def load(name_ap, shape):
                t = pers.tile(shape, F32, tag=f"w_{name_ap.name}",
                              name=f"sb_{name_ap.name}")
                nc.scalar.dma_start(t[:], name_ap[:])
                return t

            lsfl_sb = load(lsfl, [128, sh])
            w = {a.name: load(a, list(a.shape)) for a in
                 (lm_w1s, lm_b1, lm_w2, lm_b2r, cm_w1s, cm_b1, cm_w2,
                  cm_b2r, cu_w1, cu_b1, cu_w2, cu_b2, lu_w1a, lu_w1b,
                  lu_b1, lu_w2, lu_b2)}

            cuin_sb = pers.tile([128, ncdim], F32, tag="cuin")   # [CsT; MTLmsgT]
            nc.scalar.dma_start(cuin_sb[0:64, :], cst[:])
            lmsg_sb = pers.tile([128, LO, 64], BD, tag="lmsg")  # natural [l, d]
            cmsg_sb = pers.tile([128, CO, 64], BD, tag="cmsg")  # natural [c, d]
            mc_sb = pers.tile([128, l_chunk], F32, tag="mc")     # [MCmsgT; zeros]
            nc.vector.memset(mc_sb[64:128, :], 0.0)

            # ---- stage B: Lmsg = MLP_Lm(Ls) on own shard (natural) ------
            for jj in range(SB):
                hp = mlpp.tile([128, 512], F32, tag="h")
                nc.tensor.matmul(hp[:], w["lm_w1s"][:],
                                 lsfl_sb[:, jj * 512:(jj + 1) * 512],
                                 start=True, stop=True)
                hs = work.tile([128, 512], F32, tag="hs")
                nc.scalar.activation(hs[:], hp[:], AF.Relu, bias=w["lm_b1"][:])
                for s in range(4):
                    op = mlpp.tile([128, 64], F32, tag="small")
                    nc.tensor.matmul(op[:], hs[:, s * 128:(s + 1) * 128],
                                     w["lm_w2"][:], start=True, stop=True)
                    nc.vector.tensor_add(lmsg_sb[:, jj * 4 + s, :], op[:],
                                         w["lm_b2r"][:])

            # ---- stage C: partial MTLmsg^T + chunked AllReduce ----------
            # Stream 2 MiB per dma_start (two 128-row blocks); stage D is
            # deferred and interleaved into pass-2 so the PE never blocks on
            # a collective while pass-1/pass-2 tiles still need consuming.
            LO2 = LO // 2
            CO2 = CO // 2
            ccouts = []
            for cc in range(CCN):
                c0 = cc * c_chunk
                accs = [accp.tile([64, 512], F32, tag=f"acc{j}",
                                  name=f"acc_c{cc}_{j}") for j in range(NJ)]
                for lo2 in range(LO2):
                    mt = stream.tile([128, 2, c_chunk], BD, tag="mstream", name=f"mtc{cc}_{lo2}")
                    src = ms[lo2 * 256:(lo2 + 1) * 256, c0:c0 + c_chunk]
                    eng = nc.sync if lo2 % 2 == 0 else nc.gpsimd
                    eng.dma_start(mt[:], src.rearrange("(k p) f -> p k f",
                                                       p=128))
                    for k in range(2):
                        for j in range(NJ):
                            nc.tensor.matmul(
                                accs[j][:], mm(lmsg_sb[:, lo2 * 2 + k, :]),
                                mm(mt[:, k, j * 512:(j + 1) * 512]),
                                start=(lo2 == 0 and k == 0),
                                stop=(lo2 == LO2 - 1 and k == 1))
                par = work.tile([64, c_chunk], F32, tag="par")
                for j in range(NJ):
                    nc.vector.tensor_copy(par[:, j * 512:(j + 1) * 512],
                                          accs[j][:])
                ccin = dram.tile([64, c_chunk], F32, tag=f"ccin{cc}")
                ccout = dram.tile([64, c_chunk], F32, tag=f"ccout{cc}")
                nc.scalar.dma_start(ccin[:], par[:])
                nc.gpsimd.collective_compute(
                    "AllReduce", mybir.AluOpType.add,
                    replica_groups=[[0, 1, 2, 3], [4, 5, 6, 7]],
                    ins=[ccin.opt()], outs=[ccout.opt()])
                ccouts.append(ccout)

            # ---- stage D (chunk cc): cu + Cmsg, emitted lazily ----------
            def stage_d(cc):
                c0 = cc * c_chunk
                nc.scalar.dma_start(cuin_sb[64:128, c0:c0 + c_chunk],
                                    ccouts[cc][:])
                for q in range(BQ):
                    col = c0 + q * 512
                    hp = mlpp.tile([128, 512], F32, tag="h", name=f"hpD{cc}{q}")
                    nc.tensor.matmul(hp[:], w["cu_w1"][:],
                                     cuin_sb[:, col:col + 512],
                                     start=True, stop=True)
                    hs = work.tile([128, 512], F32, tag="hs", name=f"hsD{cc}{q}")
                    nc.scalar.activation(hs[:], hp[:], AF.Relu,
                                         bias=w["cu_b1"][:])
                    tp = mlpp.tile([64, 512], F32, tag="small",
                                   name=f"tpD{cc}{q}")
                    nc.tensor.matmul(tp[:], w["cu_w2"][:], hs[:],
                                     start=True, stop=True)
                    ts_ = work.tile([64, 512], F32, tag="ts", name=f"tsD{cc}{q}")
                    nc.scalar.activation(ts_[:], tp[:], AF.Tanh,
                                         bias=w["cu_b2"][:])
                    # cu^T overwrites the (now dead) Cs^T rows in place
                    nc.vector.tensor_add(cuin_sb[0:64, col:col + 512],
                                         cuin_sb[0:64, col:col + 512], ts_[:])
                    nc.scalar.dma_start(cut_o[:, col:col + 512],
                                        cuin_sb[0:64, col:col + 512])
                    h2p = mlpp.tile([128, 512], F32, tag="h", name=f"h2pD{cc}{q}")
                    nc.tensor.matmul(h2p[:], w["cm_w1s"][:],
                                     cuin_sb[:, col:col + 512],
                                     start=True, stop=True)
                    h2s = work.tile([128, 512], F32, tag="hs",
                                    name=f"h2sD{cc}{q}")
                    nc.scalar.activation(h2s[:], h2p[:], AF.Relu,
                                         bias=w["cm_b1"][:])
                    for t in range(4):
                        op = mlpp.tile([128, 64], F32, tag="small",
                                       name=f"opD{cc}{q}{t}")
                        nc.tensor.matmul(op[:], h2s[:, t * 128:(t + 1) * 128],
                                         w["cm_w2"][:], start=True, stop=True)
                        nc.vector.tensor_add(cmsg_sb[:, col // 128 + t, :],
                                             op[:], w["cm_b2r"][:])

            # ---- stage E: MCmsg^T + lu; stage-D chunks interleaved ------
            d_step = max(1, CO2 // CCN)
            if sh % 4096 == 0 and l_chunk == 2048:
                e_chunks = [2048] * (sh // 2048 - 1) + [1536, 512]
            else:
                e_chunks = [l_chunk] * LCN
            e_starts = [sum(e_chunks[:i]) for i in range(len(e_chunks))]
            for lc in range(len(e_chunks)):
                lck = e_chunks[lc]
                lcj = lck // 512
                l0 = e_starts[lc]
                accs = [accp.tile([64, 512], F32, tag=f"acc{j}",
                                  name=f"acc_l{lc}_{j}") for j in range(lcj)]
                for co2 in range(CO2):
                    if lc == 0 and co2 % d_step == 0 and co2 // d_step < CCN:
                        stage_d(co2 // d_step)
                    mt = stream.tile([128, 2, l_chunk], BD, tag="mstream", name=f"mte{lc}_{co2}")[:, :, :lck]
                    src = mst[co2 * 256:(co2 + 1) * 256, l0:l0 + lck]
                    eng = nc.sync if co2 % 2 == 0 else nc.gpsimd
                    eng.dma_start(mt[:], src.rearrange("(k p) f -> p k f",
                                                       p=128))
                    for k in range(2):
                        for j in range(lcj):
                            nc.tensor.matmul(
                                accs[j][:], mm(cmsg_sb[:, co2 * 2 + k, :]),
                                mm(mt[:, k, j * 512:(j + 1) * 512]),
                                start=(co2 == 0 and k == 0),
                                stop=(co2 == CO2 - 1 and k == 1))
                for j in range(lcj):
                    nc.vector.tensor_copy(mc_sb[0:64, j * 512:(j + 1) * 512],
                                          accs[j][:])
                for j in range(lcj):
                    sl = l0 + j * 512
                    hp = mlpp.tile([128, 512], F32, tag="h", name=f"hpE{lc}{j}")
                    nc.tensor.matmul(hp[:], w["lu_w1a"][:],
                                     lsfl_sb[:, sl:sl + 512],
                                     start=True, stop=False)
                    nc.tensor.matmul(hp[:], w["lu_w1b"][:],
                                     mc_sb[:, j * 512:(j + 1) * 512],
                                     start=False, stop=True)
                    hs = work.tile([128, 512], F32, tag="hs", name=f"hsE{lc}{j}")
                    nc.scalar.activation(hs[:], hp[:], AF.Relu,
                                         bias=w["lu_b1"][:])
                    tp = mlpp.tile([64, 512], F32, tag="small",
                                   name=f"tpE{lc}{j}")
                    nc.tensor.matmul(tp[:], w["lu_w2"][:], hs[:],
                                     start=True, stop=True)
                    ts_ = work.tile([64, 512], F32, tag="ts", name=f"tsE{lc}{j}")
                    nc.scalar.activation(ts_[:], tp[:], AF.Tanh,
                                         bias=w["lu_b2"][:])
                    lut_sb = work.tile([64, 512], F32, tag="lut",
                                       name=f"lutE{lc}{j}")
                    nc.vector.tensor_add(lut_sb[:], lsfl_sb[0:64, sl:sl + 512],
                                         ts_[:])
                    nc.scalar.dma_start(lut_o[:, sl:sl + 512], lut_sb[:])

    nc.compile()
    return nc


def make_in_maps(Ls, Cs, Ms, Lm_w1, Lm_b1, Lm_w2, Lm_b2, Cm_w1, Cm_b1,
                 Cm_w2, Cm_b2, Cu_w1, Cu_b1, Cu_w2, Cu_b2, Lu_w1, Lu_b1,
                 Lu_w2, Lu_b2, sh=SH):
    f = lambda a: np.ascontiguousarray(np.asarray(a, dtype=np.float32))
    Ls, Cs, Ms = f(Ls), f(Cs), f(Ms)
    b, nl, d = Ls.shape
    ncdim = Cs.shape[1]
    z64 = np.zeros((64, 128), np.float32)
    weights = {
        "lm_w1s": np.concatenate([f(Lm_w1), z64], 0),
        "lm_b1": f(Lm_b1).reshape(128, 1),
        "lm_w2": f(Lm_w2),
        "lm_b2r": np.broadcast_to(f(Lm_b2).reshape(1, 64), (128, 64)).copy(),
        "cm_w1s": np.concatenate([f(Cm_w1), z64], 0),
        "cm_b1": f(Cm_b1).reshape(128, 1),
        "cm_w2": f(Cm_w2),
        "cm_b2r": np.broadcast_to(f(Cm_b2).reshape(1, 64), (128, 64)).copy(),
        "cu_w1": f(Cu_w1),
        "cu_b1": f(Cu_b1).reshape(128, 1),
        "cu_w2": f(Cu_w2),
        "cu_b2": f(Cu_b2).reshape(64, 1),
        "lu_w1a": f(Lu_w1)[0:128],
        "lu_w1b": np.concatenate([f(Lu_w1)[128:192], z64], 0),
        "lu_b1": f(Lu_b1).reshape(128, 1),
        "lu_w2": f(Lu_w2),
        "lu_b2": f(Lu_b2).reshape(64, 1),
    }
    in_maps = []
    for core in range(N_CORES):
        g, i = divmod(core, 4)
        blk = Ms[g, i * sh:(i + 1) * sh, :]
        lsT = Ls[g].T
        flT = lsT.reshape(d, nl // 2, 2)[:, :, ::-1].reshape(d, nl)
        lsfl = np.concatenate(
            [lsT[:, i * sh:(i + 1) * sh], flT[:, i * sh:(i + 1) * sh]], 0)
        in_maps.append({
            "ms": np.ascontiguousarray(blk),
            "mst": np.ascontiguousarray(blk.T),
            "lsfl": np.ascontiguousarray(lsfl),
            "cst": np.ascontiguousarray(Cs[g].T),
            **weights,
        })
    return in_maps


def kernel(Ls, Cs, Ms, Lm_w1, Lm_b1, Lm_w2, Lm_b2, Cm_w1, Cm_b1, Cm_w2,
           Cm_b2, Cu_w1, Cu_b1, Cu_w2, Cu_b2, Lu_w1, Lu_b1, Lu_w2, Lu_b2):
    global LAST_RESULT
    key = BIG_DT
    if key not in _CACHE:
        _CACHE[key] = build_nc(big_dt=BIG_DT)
    nc = _CACHE[key]
    in_maps = make_in_maps(Ls, Cs, Ms, Lm_w1, Lm_b1, Lm_w2, Lm_b2, Cm_w1,
                           Cm_b1, Cm_w2, Cm_b2, Cu_w1, Cu_b1, Cu_w2, Cu_b2,
                           Lu_w1, Lu_b1, Lu_w2, Lu_b2)
    kwargs = {}
    if TRACE:
        kwargs = {"trace": True, "tmpdir": TRACE_DIR}
    res = run_bass_kernel_spmd(nc, in_maps, list(range(N_CORES)), **kwargs)
    LAST_RESULT = res
    b, nl, d = np.asarray(Ls).shape
    ncdim = np.asarray(Cs).shape[1]
    lu = np.empty((b, nl, d), np.float32)
    cu = np.empty((b, ncdim, d), np.float32)
    for core in range(N_CORES):
        g, i = divmod(core, 4)
        lu[g, i * SH:(i + 1) * SH] = res.results[core]["lut"].T
    cu[0] = res.results[0]["cut"].T
    cu[1] = res.results[4]["cut"].T
    return lu, cu


# revision 8
# speedup vs baseline: 1.1368x; 1.1368x over previous
"""Trainium2 Bass kernel for the BiPartialTestBlock GNN message-passing layer.

Math (per batch g):
    Lmsg   = MLP_Lm(Ls)                                  [nL, d]
    MTLmsg = Ms^T @ Lmsg                                 [nC, d]
    cu     = Cs + tanh(MLP_Cu([Cs, MTLmsg]))             [nC, d]
    Cmsg   = MLP_Cm(cu)                                  [nC, d]
    MCmsg  = Ms @ Cmsg                                   [nL, d]
    lu     = Ls + tanh(MLP_Lu([Ls, FL, MCmsg]))          [nL, d]
returns (lu, cu).

Sharding (8 cores): core c -> (g = c // 4) batch, (i = c % 4) nL-shard of
4096 rows. Pass 1 contracts over each core's nL shard (partial MTLmsg),
AllReduce'd over the 4-core group in 4 pipelined nC chunks. cu/Cmsg are
recomputed (replicated) per core; pass 2 uses a host-transposed copy of the
core's own Ms block, so both big matmuls stream natural-layout [128, 2048]
f32 tiles at full DMA width. All activations live feature-major ([d, tok]),
which makes every MLP a weights-stationary matmul and the two concats pure
partition-stacking.
"""

import sys

sys.path.insert(0, "/opt/trn_rl_repo")

import numpy as np

import concourse.bass as bass  # noqa: F401  (AP types via bacc)
import concourse.mybir as mybir
import concourse.tile as tile
from concourse import bacc
from concourse.bass_utils import run_bass_kernel_spmd

N_CORES = 8
B, NL, NC, D, H = 2, 16384, 8192, 64, 128
SH = NL // 4  # per-core literal shard

F32 = mybir.dt.float32
F32R = mybir.dt.float32r
AF = mybir.ActivationFunctionType

# "f32r" runs the two adjacency matmuls in reduced-precision fp32 (1 PE
# cycle/row instead of 4); "f32" is full precision.
BIG_DT = "f32r"
TRACE = False
TRACE_DIR = None
LAST_RESULT = None

_CACHE = {}


def build_nc(big_dt=BIG_DT, sh=SH, ncdim=NC, c_chunk=2048, l_chunk=2048):
    """Emit the SPMD Bass module (identical program on all 8 cores)."""
    c_chunk = min(c_chunk, ncdim)
    l_chunk = min(l_chunk, sh)
    LO = sh // 128            # pass-1 l-tiles (contraction)
    CCN = ncdim // c_chunk    # pass-1 / allreduce chunks
    NJ = c_chunk // 512       # psum banks per pass-1 chunk
    CO = ncdim // 128         # pass-2 c-tiles (contraction)
    LCN = sh // l_chunk       # pass-2 chunks
    LJ = l_chunk // 512
    QN = ncdim // 512         # cu/Cmsg sub-chunks
    BQ = c_chunk // 512       # stage-D sub-chunks per allreduce chunk
    SB = sh // 512            # Lmsg sub-chunks

    BD = F32R if big_dt == "f32r" else F32

    def mm(ap):
        return ap

    nc = bacc.Bacc("TRN2", target_bir_lowering=False, debug=False,
                   enable_asserts=False, num_devices=N_CORES)

    def din(name, shape):
        return nc.dram_tensor(name, shape, F32, kind="ExternalInput").ap()

    ms = nc.dram_tensor("ms", [sh, ncdim], BD, kind="ExternalInput").ap()
    mst = nc.dram_tensor("mst", [ncdim, sh], BD, kind="ExternalInput").ap()
    lsfl = din("lsfl", [128, sh])
    cst = din("cst", [64, ncdim])
    lm_w1s = din("lm_w1s", [128, 128])
    lm_b1 = din("lm_b1", [128, 1])
    lm_w2 = din("lm_w2", [128, 64])
    lm_b2r = din("lm_b2r", [128, 64])
    cm_w1s = din("cm_w1s", [128, 128])
    cm_b1 = din("cm_b1", [128, 1])
    cm_w2 = din("cm_w2", [128, 64])
    cm_b2r = din("cm_b2r", [128, 64])
    cu_w1 = din("cu_w1", [128, 128])
    cu_b1 = din("cu_b1", [128, 1])
    cu_w2 = din("cu_w2", [128, 64])
    cu_b2 = din("cu_b2", [64, 1])
    lu_w1a = din("lu_w1a", [128, 128])
    lu_w1b = din("lu_w1b", [128, 128])
    lu_b1 = din("lu_b1", [128, 1])
    lu_w2 = din("lu_w2", [128, 64])
    lu_b2 = din("lu_b2", [64, 1])

    lut_o = nc.dram_tensor("lut", [64, sh], F32, kind="ExternalOutput").ap()
    cut_o = nc.dram_tensor("cut", [64, ncdim], F32, kind="ExternalOutput").ap()

    with tile.TileContext(nc) as tc:
        with tc.tile_pool(name="pers", bufs=1) as pers, \
             tc.tile_pool(name="stream", bufs=5) as stream, \
             tc.tile_pool(name="work", bufs=3) as work, \
             tc.tile_pool(name="accp", bufs=1, space="PSUM") as accp, \
             tc.tile_pool(name="mlpp", bufs=2, space="PSUM") as mlpp, \
             tc.tile_pool(name="dram", bufs=1, space="DRAM") as dram:

            # ---- prologue: small inputs -> SBUF -------------------------
            def load(name_ap, shape):
                t = pers.tile(shape, F32, tag=f"w_{name_ap.name}",
                              name=f"sb_{name_ap.name}")
                nc.scalar.dma_start(t[:], name_ap[:])
                return t

            lsfl_sb = load(lsfl, [128, sh])
            w = {a.name: load(a, list(a.shape)) for a in
                 (lm_w1s, lm_b1, lm_w2, lm_b2r, cm_w1s, cm_b1, cm_w2,
                  cm_b2r, cu_w1, cu_b1, cu_w2, cu_b2, lu_w1a, lu_w1b,
                  lu_b1, lu_w2, lu_b2)}

            cuin_sb = pers.tile([128, ncdim], F32, tag="cuin")   # [CsT; MTLmsgT]
            nc.scalar.dma_start(cuin_sb[0:64, :], cst[:])
            lmsg_sb = pers.tile([128, LO, 64], BD, tag="lmsg")  # natural [l, d]
            cmsg_sb = pers.tile([128, CO, 64], BD, tag="cmsg")  # natural [c, d]
            mc_sb = pers.tile([128, l_chunk], F32, tag="mc")     # [MCmsgT; zeros]
            nc.vector.memset(mc_sb[64:128, :], 0.0)

            # ---- stage B: Lmsg = MLP_Lm(Ls) on own shard (natural) ------
            for jj in range(SB):
                hp = mlpp.tile([128, 512], F32, tag="h")
                nc.tensor.matmul(hp[:], w["lm_w1s"][:],
                                 lsfl_sb[:, jj * 512:(jj + 1) * 512],
                                 start=True, stop=True)
                hs = work.tile([128, 512], F32, tag="hs")
                nc.scalar.activation(hs[:], hp[:], AF.Relu, bias=w["lm_b1"][:])
                for s in range(4):
                    op = mlpp.tile([128, 64], F32, tag="small")
                    nc.tensor.matmul(op[:], hs[:, s * 128:(s + 1) * 128],
                                     w["lm_w2"][:], start=True, stop=True)
                    nc.vector.tensor_add(lmsg_sb[:, jj * 4 + s, :], op[:],
                                         w["lm_b2r"][:])

            # ---- stage C: partial MTLmsg^T + chunked AllReduce ----------
            # Stream 2 MiB per dma_start (two 128-row blocks); stage D is
            # deferred and interleaved into pass-2 so the PE never blocks on
            # a collective while pass-1/pass-2 tiles still need consuming.
            LO2 = LO // 2
            CO2 = CO // 2
            ccouts = []
            for cc in range(CCN):
                c0 = cc * c_chunk
                accs = [accp.tile([64, 512], F32, tag=f"acc{j}",
                                  name=f"acc_c{cc}_{j}") for j in range(NJ)]
                for lo2 in range(LO2):
                    mt = stream.tile([128, 2, c_chunk], BD, tag="mstream", name=f"mtc{cc}_{lo2}")
                    src = ms[lo2 * 256:(lo2 + 1) * 256, c0:c0 + c_chunk]
                    nc.sync.dma_start(mt[:], src.rearrange("(k p) f -> p k f",
                                                           p=128))
                    for k in range(2):
                        for j in range(NJ):
                            nc.tensor.matmul(
                                accs[j][:], mm(lmsg_sb[:, lo2 * 2 + k, :]),
                                mm(mt[:, k, j * 512:(j + 1) * 512]),
                                start=(lo2 == 0 and k == 0),
                                stop=(lo2 == LO2 - 1 and k == 1))
                par = work.tile([64, c_chunk], F32, tag="par")
                for j in range(NJ):
                    nc.vector.tensor_copy(par[:, j * 512:(j + 1) * 512],
                                          accs[j][:])
                ccin = dram.tile([64, c_chunk], F32, tag=f"ccin{cc}")
                ccout = dram.tile([64, c_chunk], F32, tag=f"ccout{cc}")
                nc.scalar.dma_start(ccin[:], par[:])
                nc.gpsimd.collective_compute(
                    "AllReduce", mybir.AluOpType.add,
                    replica_groups=[[0, 1, 2, 3], [4, 5, 6, 7]],
                    ins=[ccin.opt()], outs=[ccout.opt()])
                ccouts.append(ccout)

            # ---- stage D (chunk cc): cu + Cmsg, emitted lazily ----------
            def stage_d(cc):
                c0 = cc * c_chunk
                nc.scalar.dma_start(cuin_sb[64:128, c0:c0 + c_chunk],
                                    ccouts[cc][:])
                for q in range(BQ):
                    col = c0 + q * 512
                    hp = mlpp.tile([128, 512], F32, tag="h", name=f"hpD{cc}{q}")
                    nc.tensor.matmul(hp[:], w["cu_w1"][:],
                                     cuin_sb[:, col:col + 512],
                                     start=True, stop=True)
                    hs = work.tile([128, 512], F32, tag="hs", name=f"hsD{cc}{q}")
                    nc.scalar.activation(hs[:], hp[:], AF.Relu,
                                         bias=w["cu_b1"][:])
                    tp = mlpp.tile([64, 512], F32, tag="small",
                                   name=f"tpD{cc}{q}")
                    nc.tensor.matmul(tp[:], w["cu_w2"][:], hs[:],
                                     start=True, stop=True)
                    ts_ = work.tile([64, 512], F32, tag="ts", name=f"tsD{cc}{q}")
                    nc.scalar.activation(ts_[:], tp[:], AF.Tanh,
                                         bias=w["cu_b2"][:])
                    # cu^T overwrites the (now dead) Cs^T rows in place
                    nc.vector.tensor_add(cuin_sb[0:64, col:col + 512],
                                         cuin_sb[0:64, col:col + 512], ts_[:])
                    nc.scalar.dma_start(cut_o[:, col:col + 512],
                                        cuin_sb[0:64, col:col + 512])
                    h2p = mlpp.tile([128, 512], F32, tag="h", name=f"h2pD{cc}{q}")
                    nc.tensor.matmul(h2p[:], w["cm_w1s"][:],
                                     cuin_sb[:, col:col + 512],
                                     start=True, stop=True)
                    h2s = work.tile([128, 512], F32, tag="hs",
                                    name=f"h2sD{cc}{q}")
                    nc.scalar.activation(h2s[:], h2p[:], AF.Relu,
                                         bias=w["cm_b1"][:])
                    for t in range(4):
                        op = mlpp.tile([128, 64], F32, tag="small",
                                       name=f"opD{cc}{q}{t}")
                        nc.tensor.matmul(op[:], h2s[:, t * 128:(t + 1) * 128],
                                         w["cm_w2"][:], start=True, stop=True)
                        nc.vector.tensor_add(cmsg_sb[:, col // 128 + t, :],
                                             op[:], w["cm_b2r"][:])

            # ---- stage E: MCmsg^T + lu; stage-D chunks interleaved ------
            d_step = max(1, CO2 // CCN)
            if sh % 4096 == 0 and l_chunk == 2048:
                e_chunks = [2048] * (sh // 2048 - 1) + [1536, 512]
            else:
                e_chunks = [l_chunk] * LCN
            e_starts = [sum(e_chunks[:i]) for i in range(len(e_chunks))]
            for lc in range(len(e_chunks)):
                lck = e_chunks[lc]
                lcj = lck // 512
                l0 = e_starts[lc]
                accs = [accp.tile([64, 512], F32, tag=f"acc{j}",
                                  name=f"acc_l{lc}_{j}") for j in range(lcj)]
                for co2 in range(CO2):
                    if lc == 0 and co2 % d_step == 0 and co2 // d_step < CCN:
                        stage_d(co2 // d_step)
                    mt = stream.tile([128, 2, l_chunk], BD, tag="mstream", name=f"mte{lc}_{co2}")[:, :, :lck]
                    src = mst[co2 * 256:(co2 + 1) * 256, l0:l0 + lck]
                    nc.sync.dma_start(mt[:], src.rearrange("(k p) f -> p k f",
                                                           p=128))
                    for k in range(2):
                        for j in range(lcj):
                            nc.tensor.matmul(
                                accs[j][:], mm(cmsg_sb[:, co2 * 2 + k, :]),
                                mm(mt[:, k, j * 512:(j + 1) * 512]),
                                start=(co2 == 0 and k == 0),
                                stop=(co2 == CO2 - 1 and k == 1))
                for j in range(lcj):
                    nc.vector.tensor_copy(mc_sb[0:64, j * 512:(j + 1) * 512],
                                          accs[j][:])
                for j in range(lcj):
                    sl = l0 + j * 512
                    hp = mlpp.tile([128, 512], F32, tag="h", name=f"hpE{lc}{j}")
                    nc.tensor.matmul(hp[:], w["lu_w1a"][:],
                                     lsfl_sb[:, sl:sl + 512],
                                     start=True, stop=False)
                    nc.tensor.matmul(hp[:], w["lu_w1b"][:],
                                     mc_sb[:, j * 512:(j + 1) * 512],
                                     start=False, stop=True)
                    hs = work.tile([128, 512], F32, tag="hs", name=f"hsE{lc}{j}")
                    nc.scalar.activation(hs[:], hp[:], AF.Relu,
                                         bias=w["lu_b1"][:])
                    tp = mlpp.tile([64, 512], F32, tag="small",
                                   name=f"tpE{lc}{j}")
                    nc.tensor.matmul(tp[:], w["lu_w2"][:], hs[:],
                                     start=True, stop=True)
                    ts_ = work.tile([64, 512], F32, tag="ts", name=f"tsE{lc}{j}")
                    nc.scalar.activation(ts_[:], tp[:], AF.Tanh,
                                         bias=w["lu_b2"][:])
                    lut_sb = work.tile([64, 512], F32, tag="lut",
                                       name=f"lutE{lc}{j}")
                    nc.vector.tensor_add(lut_sb[:], lsfl_sb[0:64, sl:sl + 512],
                                         ts_[:])
                    nc.scalar.dma_start(lut_o[:, sl:sl + 512], lut_sb[:])

    nc.compile()
    return nc


def make_in_maps(Ls, Cs, Ms, Lm_w1, Lm_b1, Lm_w2, Lm_b2, Cm_w1, Cm_b1,
                 Cm_w2, Cm_b2, Cu_w1, Cu_b1, Cu_w2, Cu_b2, Lu_w1, Lu_b1,
                 Lu_w2, Lu_b2, sh=SH):
    f = lambda a: np.ascontiguousarray(np.asarray(a, dtype=np.float32))
    Ls, Cs, Ms = f(Ls), f(Cs), f(Ms)
    b, nl, d = Ls.shape
    ncdim = Cs.shape[1]
    z64 = np.zeros((64, 128), np.float32)
    weights = {
        "lm_w1s": np.concatenate([f(Lm_w1), z64], 0),
        "lm_b1": f(Lm_b1).reshape(128, 1),
        "lm_w2": f(Lm_w2),
        "lm_b2r": np.broadcast_to(f(Lm_b2).reshape(1, 64), (128, 64)).copy(),
        "cm_w1s": np.concatenate([f(Cm_w1), z64], 0),
        "cm_b1": f(Cm_b1).reshape(128, 1),
        "cm_w2": f(Cm_w2),
        "cm_b2r": np.broadcast_to(f(Cm_b2).reshape(1, 64), (128, 64)).copy(),
        "cu_w1": f(Cu_w1),
        "cu_b1": f(Cu_b1).reshape(128, 1),
        "cu_w2": f(Cu_w2),
        "cu_b2": f(Cu_b2).reshape(64, 1),
        "lu_w1a": f(Lu_w1)[0:128],
        "lu_w1b": np.concatenate([f(Lu_w1)[128:192], z64], 0),
        "lu_b1": f(Lu_b1).reshape(128, 1),
        "lu_w2": f(Lu_w2),
        "lu_b2": f(Lu_b2).reshape(64, 1),
    }
    in_maps = []
    for core in range(N_CORES):
        g, i = divmod(core, 4)
        blk = Ms[g, i * sh:(i + 1) * sh, :]
        lsT = Ls[g].T
        flT = lsT.reshape(d, nl // 2, 2)[:, :, ::-1].reshape(d, nl)
        lsfl = np.concatenate(
            [lsT[:, i * sh:(i + 1) * sh], flT[:, i * sh:(i + 1) * sh]], 0)
        in_maps.append({
            "ms": np.ascontiguousarray(blk),
            "mst": np.ascontiguousarray(blk.T),
            "lsfl": np.ascontiguousarray(lsfl),
            "cst": np.ascontiguousarray(Cs[g].T),
            **weights,
        })
    return in_maps


def kernel(Ls, Cs, Ms, Lm_w1, Lm_b1, Lm_w2, Lm_b2, Cm_w1, Cm_b1, Cm_w2,
           Cm_b2, Cu_w1, Cu_b1, Cu_w2, Cu_b2, Lu_w1, Lu_b1, Lu_w2, Lu_b2):
    global LAST_RESULT
    key = BIG_DT
    if key not in _CACHE:
        _CACHE[key] = build_nc(big_dt=BIG_DT)
    nc = _CACHE[key]
    in_maps = make_in_maps(Ls, Cs, Ms, Lm_w1, Lm_b1, Lm_w2, Lm_b2, Cm_w1,
                           Cm_b1, Cm_w2, Cm_b2, Cu_w1, Cu_b1, Cu_w2, Cu_b2,
                           Lu_w1, Lu_b1, Lu_w2, Lu_b2)
    kwargs = {}
    if TRACE:
        kwargs = {"trace": True, "tmpdir": TRACE_DIR}
    res = run_bass_kernel_spmd(nc, in_maps, list(range(N_CORES)), **kwargs)
    LAST_RESULT = res
    b, nl, d = np.asarray(Ls).shape
    ncdim = np.asarray(Cs).shape[1]
    lu = np.empty((b, nl, d), np.float32)
    cu = np.empty((b, ncdim, d), np.float32)
    for core in range(N_CORES):
        g, i = divmod(core, 4)
        lu[g, i * SH:(i + 1) * SH] = res.results[core]["lut"].T
    cu[0] = res.results[0]["cut"].T
    cu[1] = res.results[4]["cut"].T
    return lu, cu
